# revision 17
# baseline (speedup 1.0000x reference)
"""Trainium2 Bass kernel: BinaryHungarianMatcherV2 cost-matrix build.

C[b,q,t] = 5*L1(pred_box, tgt_box) + 2*focal_class(q) + 2*(-giou)
masked to 1e9 where t >= num_boxes[b].

Sharding: batch dim (32) split across 8 NeuronCores, 4 batches/core (pure
data parallel, no collectives). Per core, the [Q=1800, T=500] cost tiles
are built as 15 q-tiles of [120, 500]: q on the partition axis, t on the
free axis.

All pairwise work runs on the DVE (vector engine) as fused custom ops
(min/max diffs, relu-products, abs-sums, one reciprocal via the single
common denominator union*area_e), with a few stock tensor_tensor ops
offloaded to GPSIMD. Everything separable (corner coords, areas, focal
class cost, validity mask) is precomputed on host in fp64 and shipped as
per-q scalar columns plus per-t rows replicated across 128 partitions.
"""

import os

import numpy as np

B, Q, T = 32, 1800, 500
N_CORES = 8
B_PER = B // N_CORES          # 4 batches per core
QT = 128                      # q-tile partition size (full partitions)
NQT = 15                      # 14 full tiles + 1 overlapping tile per batch

# replicated per-target rows (each broadcast to 128 partitions)
R_X0, R_Y0, R_X1, R_Y1, R_CX, R_CY, R_W, R_H, R_A2, R_MK = range(10)
NREP = 10
# per-query scalar columns
S_X0, S_Y0, S_X1, S_Y1, S_CX, S_CY, S_W, S_H, S_A1, S_CC = range(10)
NSC = 10

INVALID = 1.0e9

_OPS = None
LAST_RESULTS = None


def _get_ops():
    """Register the fused custom-DVE ops (idempotent). Returns dict name->DveOp."""
    global _OPS
    if _OPS is not None:
        return _OPS
    from concourse import dve_ops
    from concourse.dve_ops import DveOp
    from concourse.dve_spec import (
        Spec, Src0, Src1, C0, C1, C2, Zero, relu, maxx, minn, select, lower,
    )
    from concourse.dve_uop import DveOpSpec

    def reg(name, spec):
        for op in dve_ops.OPS:
            if op.name == name:
                return op
        row = max(dve_ops._SUB_OPCODE_FOR_NAME.values()) + 1
        assert row < 0x20, "custom-DVE opcode rows exhausted"
        dve_ops._SUB_OPCODE_FOR_NAME[name] = row
        shas = {}
        for ver in ("v3", "v4"):
            s = DveOpSpec(name=name, opcode=row, uops=lower(spec, ver=ver),
                          rd1_en=dve_ops.has_src1(spec))
            shas[ver] = s.sha(ver)
        op = DveOp(name, spec, subdim=False, uops_sha=shas)
        dve_ops.OPS.append(op)
        dve_ops.CUSTOM_DVE_SPECS[name] = spec
        return op

    _d0 = Src0 - C0
    _d1 = Src1 - C1
    _OPS = {
        # wd = min(x1_t, x1_q) - max(x0_t, x0_q)
        "BHM_IDIFF": reg("BHM_IDIFF", Spec(
            body=minn(Src0, C0) - maxx(Src1, C1),
            reference=lambda in0, in1, s0, s1, imm2:
                np.minimum(in0, s0) - np.maximum(in1, s1))),
        # we = max(x1_t, x1_q) - min(x0_t, x0_q)
        "BHM_EDIFF": reg("BHM_EDIFF", Spec(
            body=maxx(Src0, C0) - minn(Src1, C1),
            reference=lambda in0, in1, s0, s1, imm2:
                np.maximum(in0, s0) - np.minimum(in1, s1))),
        # inter = relu(wd) * relu(hd)
        "BHM_RELUMUL": reg("BHM_RELUMUL", Spec(
            body=relu(Src0) * relu(Src1),
            reference=lambda in0, in1, s0, s1, imm2:
                np.maximum(in0, 0) * np.maximum(in1, 0))),
        # union = (a2_t * 1 + a1_q) + inter * (-1); generic 2-tensor FMA
        "BHM_FMA3": reg("BHM_FMA3", Spec(
            body=(Src0 * C2 + C0) + Src1 * C1,
            reference=lambda in0, in1, s0, s1, imm2:
                (in0 * imm2 + s0) + in1 * s1)),
        # N = union^2 + inter*area_e
        "BHM_SQADD": reg("BHM_SQADD", Spec(
            body=Src0 * Src0 + Src1,
            reference=lambda in0, in1, s0, s1, imm2: in0 * in0 + in1)),
        # gq = (N * r) * (-2)
        "BHM_MULSC": reg("BHM_MULSC", Spec(
            body=(Src0 * Src1) * C2,
            reference=lambda in0, in1, s0, s1, imm2: (in0 * in1) * imm2)),
        # a12 = |cx_t - cx_q| + |cy_t - cy_q|
        "BHM_ABS2": reg("BHM_ABS2", Spec(
            body=maxx(_d0, Zero - _d0) + maxx(_d1, Zero - _d1),
            reference=lambda in0, in1, s0, s1, imm2:
                np.abs(in0 - s0) + np.abs(in1 - s1))),
        # s = (a12 + a34) * 5 + cc2_q
        "BHM_SCOMB": reg("BHM_SCOMB", Spec(
            body=(Src0 + Src1) * C2 + C0,
            reference=lambda in0, in1, s0, s1, imm2: (in0 + in1) * imm2 + s0)),
        # C = mask ? cv : 1e9
        "BHM_MASKSEL": reg("BHM_MASKSEL", Spec(
            body=select(Src1, Src0, C2),
            reference=lambda in0, in1, s0, s1, imm2:
                np.where(in1 != 0, in0, imm2))),
    }
    return _OPS


def _build_program():
    from contextlib import ExitStack

    import concourse.bass as bass
    from concourse import mybir

    ops = _get_ops()
    f32 = mybir.dt.float32
    aluop = mybir.AluOpType
    nc = bass.Bass("TRN2",
                   use_seq_codegen=bool(int(os.environ.get("BHM_SEQCG", "0"))))

    trep = nc.dram_tensor("trep", [B_PER, NREP, 128, T], f32, kind="ExternalInput").ap()
    qsc = nc.dram_tensor("qsc", [B_PER, QT, NQT * NSC], f32, kind="ExternalInput").ap()
    cout = nc.dram_tensor("C", [B_PER, Q, T], f32, kind="ExternalOutput").ap()

    NCO = 4          # output buffer slots
    NT = B_PER * NQT  # 60 tiles
    NT = min(NT, int(os.environ.get("BHM_NT", NT)))
    REPEAT = int(os.environ.get("BHM_REPEAT", "1"))
    TF = min(T, int(os.environ.get("BHM_TFREE", T)))
    NODRAIN = bool(int(os.environ.get("BHM_NODRAIN", "0")))  # timing expt only
    STOCKONLY = bool(int(os.environ.get("BHM_STOCKONLY", "0")))  # timing expt only
    N_IN_DMAS = B_PER * NREP + B_PER

    with ExitStack() as ctx:
        rep = {}
        for b in range(B_PER):
            for r in range(NREP):
                rep[(b, r)] = ctx.enter_context(
                    nc.sbuf_tensor(f"rep_{b}_{r}", [128, T], f32))
        qs = [ctx.enter_context(nc.sbuf_tensor(f"qs_{b}", [QT, NQT * NSC], f32))
              for b in range(B_PER)]
        names = ["wd", "hd", "we", "he", "inter", "areae", "union", "n1",
                 "nn", "dd", "rr", "gq", "a12", "a34", "sp", "cv"]
        wkt = {n: ctx.enter_context(nc.sbuf_tensor(f"wk_{n}", [QT, TF], f32))
               for n in names}
        co = [ctx.enter_context(nc.sbuf_tensor(f"co_{i}", [QT, TF], f32))
              for i in range(NCO)]

        in_sem = ctx.enter_context(nc.semaphore("in_sem"))
        dve_sem = ctx.enter_context(nc.semaphore("dve_sem"))
        out_sems = [ctx.enter_context(nc.semaphore(f"out_sem_{i}"))
                    for i in range(NCO)]
        block = ctx.enter_context(nc.Block())

        @block.sync
        def _(sync):
            for b in range(B_PER):
                for r in range(NREP):
                    sync.dma_start(out=rep[(b, r)][:], in_=trep[b, r]).then_inc(in_sem, 16)
                sync.dma_start(out=qs[b][:], in_=qsc[b]).then_inc(in_sem, 16)
            for k in range(NT * REPEAT):
                b, qt = divmod(k % NT, NQT)
                q0 = min(qt * QT, Q - QT)
                r0 = QT - 8 if qt == NQT - 1 else 0
                sync.wait_ge(dve_sem, k + 1)
                sync.dma_start(out=cout[b, q0 + r0:q0 + QT, 0:TF],
                               in_=co[k % NCO][r0:QT, :]).then_inc(out_sems[k % NCO], 16)

        @block.vector
        def _(v):
            cd = v._custom_dve
            if STOCKONLY:
                def cd(op, out, in0, in1=None, s0=None, s1=None, imm2=0.0):
                    v.tensor_tensor(out, in0, in1 if in1 is not None else in0,
                                    op=aluop.mult)
            _real_drain = v.drain
            if NODRAIN:
                v = type("V", (), {})()  # shim: forward everything but drain
                for m in ("wait_ge", "tensor_tensor", "reciprocal_approx_fast"):
                    setattr(v, m, getattr(nc.vector, m))
                v.drain = lambda *a, **k: None
                cd = nc.vector._custom_dve
            v.wait_ge(in_sem, 16 * N_IN_DMAS)
            for k in range(NT * REPEAT):
                b, qt = divmod(k % NT, NQT)

                def rp(r):
                    return rep[(b, r)][0:QT, 0:TF]

                def sc(s):
                    c = qt * NSC + s
                    return qs[b][:, c:c + 1]

                def w(n):
                    return wkt[n][:]

                # L0: depends only on resident inputs
                cd(ops["BHM_IDIFF"], out=w("wd"), in0=rp(R_X1), in1=rp(R_X0),
                   s0=sc(S_X1), s1=sc(S_X0))
                cd(ops["BHM_IDIFF"], out=w("hd"), in0=rp(R_Y1), in1=rp(R_Y0),
                   s0=sc(S_Y1), s1=sc(S_Y0))
                cd(ops["BHM_EDIFF"], out=w("we"), in0=rp(R_X1), in1=rp(R_X0),
                   s0=sc(S_X1), s1=sc(S_X0))
                cd(ops["BHM_EDIFF"], out=w("he"), in0=rp(R_Y1), in1=rp(R_Y0),
                   s0=sc(S_Y1), s1=sc(S_Y0))
                cd(ops["BHM_ABS2"], out=w("a12"), in0=rp(R_CX), in1=rp(R_CY),
                   s0=sc(S_CX), s1=sc(S_CY))
                cd(ops["BHM_ABS2"], out=w("a34"), in0=rp(R_W), in1=rp(R_H),
                   s0=sc(S_W), s1=sc(S_H))
                v.drain()
                # L1
                cd(ops["BHM_RELUMUL"], out=w("inter"), in0=w("wd"), in1=w("hd"))
                v.tensor_tensor(w("areae"), w("we"), w("he"), op=aluop.mult)
                cd(ops["BHM_SCOMB"], out=w("sp"), in0=w("a12"), in1=w("a34"),
                   s0=sc(S_CC), imm2=5.0)
                v.drain()
                # L2
                cd(ops["BHM_FMA3"], out=w("union"), in0=rp(R_A2), in1=w("inter"),
                   s0=sc(S_A1), s1=-1.0, imm2=1.0)
                v.tensor_tensor(w("n1"), w("inter"), w("areae"), op=aluop.mult)
                v.drain()
                # L3
                cd(ops["BHM_SQADD"], out=w("nn"), in0=w("union"), in1=w("n1"))
                v.tensor_tensor(w("dd"), w("union"), w("areae"), op=aluop.mult)
                v.drain()
                # L4
                v.reciprocal_approx_fast(out=w("rr"), in_=w("dd"))
                v.drain()
                # L5
                cd(ops["BHM_MULSC"], out=w("gq"), in0=w("nn"), in1=w("rr"),
                   imm2=-2.0)
                v.drain()
                # L6
                v.tensor_tensor(w("cv"), w("sp"), w("gq"), op=aluop.add)
                v.drain()
                # L7
                if k >= NCO:
                    v.wait_ge(out_sems[k % NCO], 16 * (k // NCO))
                cd(ops["BHM_MASKSEL"], out=co[k % NCO][:], in0=w("cv"),
                   in1=rp(R_MK), imm2=INVALID)
                if NODRAIN:
                    nc.vector.sem_inc(dve_sem, 1)
                else:
                    v.drain().then_inc(dve_sem, 1)

    # Raw Bass skips Bacc.compile()'s codegen_inst_isa_subclasses pass;
    # without it InstCustomDveAnt .instr stays empty and walrus rejects the
    # NEFF with "ISA wrong length".
    mybir.codegen_inst_isa_subclasses(nc)
    return nc


def _host_prep(pred_logits, pred_boxes, boxes_padded, num_boxes):
    """Per-core input maps: separable terms computed on host in fp64."""
    pl = np.asarray(pred_logits, np.float64)[..., 0]          # [B,Q]
    pb = np.asarray(pred_boxes, np.float64)                   # [B,Q,4]
    tb = np.asarray(boxes_padded, np.float64)                 # [B,T,4]
    nb = np.asarray(num_boxes).astype(np.int64)               # [B]

    cx, cy, w, h = pb[..., 0], pb[..., 1], pb[..., 2], pb[..., 3]
    x0q, y0q = cx - 0.5 * w, cy - 0.5 * h
    x1q, y1q = cx + 0.5 * w, cy + 0.5 * h
    area1 = (x1q - x0q) * (y1q - y0q)

    p = 1.0 / (1.0 + np.exp(-pl))
    log_p = -np.log1p(np.exp(-pl))
    log_1mp = -np.log1p(np.exp(pl))
    cc = -0.25 * (1.0 - p) ** 2 * log_p + 0.75 * p ** 2 * log_1mp
    cc2 = 2.0 * cc + 2.0

    tcx, tcy, tw, th = tb[..., 0], tb[..., 1], tb[..., 2], tb[..., 3]
    tx0, ty0 = tcx - 0.5 * tw, tcy - 0.5 * th
    tx1, ty1 = tcx + 0.5 * tw, tcy + 0.5 * th
    area2 = (tx1 - tx0) * (ty1 - ty0)
    mk = (np.arange(T)[None, :] < nb[:, None]).astype(np.float64)   # [B,T]

    trows = np.stack([tx0, ty0, tx1, ty1, tcx, tcy, tw, th, area2, mk],
                     axis=1)                                   # [B,NREP,T]
    qcols = np.stack([x0q, y0q, x1q, y1q, cx, cy, w, h, area1, cc2],
                     axis=2)                                   # [B,Q,NSC]

    in_maps = []
    for c in range(N_CORES):
        bs = slice(c * B_PER, (c + 1) * B_PER)
        trep = np.broadcast_to(trows[bs, :, None, :], (B_PER, NREP, 128, T))
        trep = np.ascontiguousarray(trep, dtype=np.float32)
        # [B_PER, Q, NSC] -> [B_PER, QT, NQT*NSC]; tile qt covers rows
        # [q0, q0+128) with the last tile overlapping (q0 = Q-128)
        qb = qcols[bs]
        tiles = [qb[:, min(qt * QT, Q - QT):min(qt * QT, Q - QT) + QT, :]
                 for qt in range(NQT)]
        qc = np.stack(tiles, axis=1)                  # [B_PER, NQT, QT, NSC]
        qc = qc.transpose(0, 2, 1, 3)
        qc = np.ascontiguousarray(qc.reshape(B_PER, QT, NQT * NSC),
                                  dtype=np.float32)
        in_maps.append({"trep": trep, "qsc": qc})
    return in_maps


def kernel(pred_logits, pred_boxes, boxes_padded, num_boxes):
    global LAST_RESULTS
    from concourse.bass_utils import run_bass_kernel_spmd

    in_maps = _host_prep(pred_logits, pred_boxes, boxes_padded, num_boxes)
    nc = _build_program()
    trace = bool(int(os.environ.get("BHM_TRACE", "0")))
    res = run_bass_kernel_spmd(nc, in_maps, list(range(N_CORES)), trace=trace)
    LAST_RESULTS = res
    out = np.concatenate(
        [np.asarray(res.results[c]["C"]).reshape(B_PER, Q, T)
         for c in range(N_CORES)], axis=0)
    return out.astype(np.float32, copy=False)


# revision 19
# speedup vs baseline: 1.4562x; 1.4562x over previous
"""Trainium2 Bass kernel: BinaryHungarianMatcherV2 cost-matrix build.

C[b,q,t] = 5*L1(pred_box, tgt_box) + 2*focal_class(q) + 2*(-giou),
masked to 1e9 where t >= num_boxes[b].

Sharding: batch dim (32) split across 8 NeuronCores (pure data parallel,
4 batch slots per core). Per core each [Q=1800, T=500] cost slab is built
as 15 q-tiles of 128 rows (the last tile overlaps and stores only its new
8 rows), q on the partition axis, t on the free axis.

All pairwise work runs on the DVE as fused custom ops (min/max corner
diffs, relu-product, abs-sums, one reciprocal via the common denominator
union*area_e). Separable terms (corners, areas, focal class cost, the
validity mask) are precomputed on host in fp64 and shipped as per-q
scalar columns plus per-t rows replicated across partitions.

Because the 8 cores share one SPMD program, per-batch valid-column
counts are handled by sorting the 32 batches by num_boxes and assigning
similar sizes to the same program slot: slot j computes only
W[j] = max over its 8 cores of num_boxes columns (~62% of full work for
uniform sizes); columns beyond W[j] are filled with 1e9 by plain DMAs
from a constant tile.
"""

import os

import numpy as np

B, Q, T = 32, 1800, 500
N_CORES = 8
B_PER = B // N_CORES          # 4 batch slots per core
QT = 128                      # q-tile partition size (full partitions)
NQT = 15                      # 14 full tiles + 1 overlapping tile per batch

# replicated per-target rows (broadcast across partitions)
R_X0, R_Y0, R_X1, R_Y1, R_CX, R_CY, R_W, R_H, R_A2, R_MK, R_FILL = range(11)
NREP = 11
# per-query scalar columns
S_X0, S_Y0, S_X1, S_Y1, S_CX, S_CY, S_W, S_H, S_A1, S_CC = range(10)
NSC = 10

INVALID = 1.0e9

_OPS = None
_PROG_CACHE = {}
LAST_RESULTS = None


def _get_ops():
    """Register the fused custom-DVE ops (idempotent). Returns dict name->DveOp."""
    global _OPS
    if _OPS is not None:
        return _OPS
    from concourse import dve_ops
    from concourse.dve_ops import DveOp
    from concourse.dve_spec import (
        Spec, Src0, Src1, C0, C1, C2, Zero, relu, maxx, minn, select, lower,
    )
    from concourse.dve_uop import DveOpSpec

    def reg(name, spec):
        for op in dve_ops.OPS:
            if op.name == name:
                return op
        row = max(dve_ops._SUB_OPCODE_FOR_NAME.values()) + 1
        assert row < 0x20, "custom-DVE opcode rows exhausted"
        dve_ops._SUB_OPCODE_FOR_NAME[name] = row
        shas = {}
        for ver in ("v3", "v4"):
            s = DveOpSpec(name=name, opcode=row, uops=lower(spec, ver=ver),
                          rd1_en=dve_ops.has_src1(spec))
            shas[ver] = s.sha(ver)
        op = DveOp(name, spec, subdim=False, uops_sha=shas)
        dve_ops.OPS.append(op)
        dve_ops.CUSTOM_DVE_SPECS[name] = spec
        return op

    _d0 = Src0 - C0
    _d1 = Src1 - C1
    _OPS = {
        # wd = min(x1_t, x1_q) - max(x0_t, x0_q)
        "BHM_IDIFF": reg("BHM_IDIFF", Spec(
            body=minn(Src0, C0) - maxx(Src1, C1),
            reference=lambda in0, in1, s0, s1, imm2:
                np.minimum(in0, s0) - np.maximum(in1, s1))),
        # we = max(x1_t, x1_q) - min(x0_t, x0_q)
        "BHM_EDIFF": reg("BHM_EDIFF", Spec(
            body=maxx(Src0, C0) - minn(Src1, C1),
            reference=lambda in0, in1, s0, s1, imm2:
                np.maximum(in0, s0) - np.minimum(in1, s1))),
        # inter = relu(wd) * relu(hd)
        "BHM_RELUMUL": reg("BHM_RELUMUL", Spec(
            body=relu(Src0) * relu(Src1),
            reference=lambda in0, in1, s0, s1, imm2:
                np.maximum(in0, 0) * np.maximum(in1, 0))),
        # union = (a2_t * 1 + a1_q) - inter; generic 2-tensor FMA
        "BHM_FMA3": reg("BHM_FMA3", Spec(
            body=(Src0 * C2 + C0) + Src1 * C1,
            reference=lambda in0, in1, s0, s1, imm2:
                (in0 * imm2 + s0) + in1 * s1)),
        # N = union^2 + inter*area_e
        "BHM_SQADD": reg("BHM_SQADD", Spec(
            body=Src0 * Src0 + Src1,
            reference=lambda in0, in1, s0, s1, imm2: in0 * in0 + in1)),
        # gq = (N * r) * (-2)
        "BHM_MULSC": reg("BHM_MULSC", Spec(
            body=(Src0 * Src1) * C2,
            reference=lambda in0, in1, s0, s1, imm2: (in0 * in1) * imm2)),
        # a12 = |cx_t - cx_q| + |cy_t - cy_q|
        "BHM_ABS2": reg("BHM_ABS2", Spec(
            body=maxx(_d0, Zero - _d0) + maxx(_d1, Zero - _d1),
            reference=lambda in0, in1, s0, s1, imm2:
                np.abs(in0 - s0) + np.abs(in1 - s1))),
        # s = (a12 + a34) * 5 + cc2_q
        "BHM_SCOMB": reg("BHM_SCOMB", Spec(
            body=(Src0 + Src1) * C2 + C0,
            reference=lambda in0, in1, s0, s1, imm2: (in0 + in1) * imm2 + s0)),
        # C = mask ? cv : 1e9
        "BHM_MASKSEL": reg("BHM_MASKSEL", Spec(
            body=select(Src1, Src0, C2),
            reference=lambda in0, in1, s0, s1, imm2:
                np.where(in1 != 0, in0, imm2))),
    }
    return _OPS


def _plan(num_boxes):
    """Sort batches by num_boxes; slot j holds sorted[8j:8j+8] (one per core).
    Returns (order[Bslots][cores] batch indices, W[Bslots] column widths)."""
    nb = np.asarray(num_boxes).astype(np.int64)
    order = np.argsort(nb, kind="stable")
    slots = order.reshape(B_PER, N_CORES)        # slot j, core c -> batch id
    W = []
    for j in range(B_PER):
        w = int(nb[slots[j]].max())
        w = min(T, w + (w & 1))                  # even width
        W.append(w)
    return slots, tuple(W)


def _build_program(W):
    from contextlib import ExitStack

    import concourse.bass as bass
    from concourse import mybir

    ops = _get_ops()
    f32 = mybir.dt.float32
    aluop = mybir.AluOpType
    nc = bass.Bass("TRN2")

    trep = nc.dram_tensor("trep", [B_PER, NREP, 128, T], f32, kind="ExternalInput").ap()
    qsc = nc.dram_tensor("qsc", [B_PER, QT, NQT * NSC], f32, kind="ExternalInput").ap()
    cout = nc.dram_tensor("C", [B_PER, Q, T], f32, kind="ExternalOutput").ap()

    NCO = 4           # output buffer slots
    NT = B_PER * NQT  # 60 tiles
    REPEAT = int(os.environ.get("BHM_REPEAT", "1"))
    N_IN_DMAS = B_PER * NREP + B_PER
    WMAX = max(W)

    with ExitStack() as ctx:
        rep = {}
        for b in range(B_PER):
            for r in range(NREP):
                rep[(b, r)] = ctx.enter_context(
                    nc.sbuf_tensor(f"rep_{b}_{r}", [128, T], f32))
        qs = [ctx.enter_context(nc.sbuf_tensor(f"qs_{b}", [QT, NQT * NSC], f32))
              for b in range(B_PER)]
        names = ["wd", "hd", "we", "he", "inter", "areae", "union", "n1",
                 "nn", "dd", "rr", "gq", "a12", "a34", "sp", "cv"]
        wkt = {n: ctx.enter_context(nc.sbuf_tensor(f"wk_{n}", [QT, WMAX], f32))
               for n in names}
        co = [ctx.enter_context(nc.sbuf_tensor(f"co_{i}", [QT, WMAX], f32))
              for i in range(NCO)]

        in_sem = ctx.enter_context(nc.semaphore("in_sem"))
        dve_sem = ctx.enter_context(nc.semaphore("dve_sem"))
        out_sems = [ctx.enter_context(nc.semaphore(f"out_sem_{i}"))
                    for i in range(NCO)]
        block = ctx.enter_context(nc.Block())

        @block.sync
        def _(sync):
            for b in range(B_PER):
                for r in range(NREP):
                    sync.dma_start(out=rep[(b, r)][:], in_=trep[b, r]).then_inc(in_sem, 16)
                sync.dma_start(out=qs[b][:], in_=qsc[b]).then_inc(in_sem, 16)
            # constant 1e9 fill of the columns no slot computes
            sync.wait_ge(in_sem, 16 * N_IN_DMAS)
            for b in range(B_PER):
                if W[b] < T:
                    fw = T - W[b]
                    for qt in range(NQT):
                        q0 = min(qt * QT, Q - QT)
                        r0 = QT - 8 if qt == NQT - 1 else 0
                        sync.dma_start(
                            out=cout[b, q0 + r0:q0 + QT, W[b]:T],
                            in_=rep[(b, R_FILL)][r0:QT, 0:fw],
                        ).then_inc(in_sem, 16)
            for k in range(NT * REPEAT):
                b, qt = divmod(k % NT, NQT)
                n = W[b]
                q0 = min(qt * QT, Q - QT)
                r0 = QT - 8 if qt == NQT - 1 else 0
                sync.wait_ge(dve_sem, k + 1)
                sync.dma_start(out=cout[b, q0 + r0:q0 + QT, 0:n],
                               in_=co[k % NCO][r0:QT, 0:n]).then_inc(out_sems[k % NCO], 16)

        @block.vector
        def _(v):
            cd = v._custom_dve
            v.wait_ge(in_sem, 16 * N_IN_DMAS)
            for k in range(NT * REPEAT):
                b, qt = divmod(k % NT, NQT)
                n = W[b]

                def rp(r):
                    return rep[(b, r)][0:QT, 0:n]

                def sc(s):
                    c = qt * NSC + s
                    return qs[b][:, c:c + 1]

                def w(nm):
                    return wkt[nm][:, 0:n]

                # L0: depends only on resident inputs
                cd(ops["BHM_IDIFF"], out=w("wd"), in0=rp(R_X1), in1=rp(R_X0),
                   s0=sc(S_X1), s1=sc(S_X0))
                cd(ops["BHM_IDIFF"], out=w("hd"), in0=rp(R_Y1), in1=rp(R_Y0),
                   s0=sc(S_Y1), s1=sc(S_Y0))
                cd(ops["BHM_EDIFF"], out=w("we"), in0=rp(R_X1), in1=rp(R_X0),
                   s0=sc(S_X1), s1=sc(S_X0))
                cd(ops["BHM_EDIFF"], out=w("he"), in0=rp(R_Y1), in1=rp(R_Y0),
                   s0=sc(S_Y1), s1=sc(S_Y0))
                cd(ops["BHM_ABS2"], out=w("a12"), in0=rp(R_CX), in1=rp(R_CY),
                   s0=sc(S_CX), s1=sc(S_CY))
                cd(ops["BHM_ABS2"], out=w("a34"), in0=rp(R_W), in1=rp(R_H),
                   s0=sc(S_W), s1=sc(S_H))
                v.drain()
                # L1
                cd(ops["BHM_RELUMUL"], out=w("inter"), in0=w("wd"), in1=w("hd"))
                v.tensor_tensor(w("areae"), w("we"), w("he"), op=aluop.mult)
                cd(ops["BHM_SCOMB"], out=w("sp"), in0=w("a12"), in1=w("a34"),
                   s0=sc(S_CC), imm2=5.0)
                v.drain()
                # L2
                cd(ops["BHM_FMA3"], out=w("union"), in0=rp(R_A2), in1=w("inter"),
                   s0=sc(S_A1), s1=-1.0, imm2=1.0)
                v.tensor_tensor(w("n1"), w("inter"), w("areae"), op=aluop.mult)
                v.drain()
                # L3
                cd(ops["BHM_SQADD"], out=w("nn"), in0=w("union"), in1=w("n1"))
                v.tensor_tensor(w("dd"), w("union"), w("areae"), op=aluop.mult)
                v.drain()
                # L4
                v.reciprocal_approx_fast(out=w("rr"), in_=w("dd"))
                v.drain()
                # L5
                cd(ops["BHM_MULSC"], out=w("gq"), in0=w("nn"), in1=w("rr"),
                   imm2=-2.0)
                v.drain()
                # L6
                v.tensor_tensor(w("cv"), w("sp"), w("gq"), op=aluop.add)
                v.drain()
                # L7: mask + write to output slot
                if k >= NCO:
                    v.wait_ge(out_sems[k % NCO], 16 * (k // NCO))
                cd(ops["BHM_MASKSEL"], out=co[k % NCO][:, 0:n], in0=w("cv"),
                   in1=rp(R_MK), imm2=INVALID)
                v.drain().then_inc(dve_sem, 1)

    # Raw Bass skips Bacc.compile()'s codegen_inst_isa_subclasses pass;
    # without it InstCustomDveAnt .instr stays empty and walrus rejects the
    # NEFF with "ISA wrong length".
    mybir.codegen_inst_isa_subclasses(nc)
    return nc


def _host_prep(pred_logits, pred_boxes, boxes_padded, num_boxes, slots):
    """Per-core input maps (separable terms in fp64); slots[j][c] = batch id."""
    pl = np.asarray(pred_logits, np.float64)[..., 0]          # [B,Q]
    pb = np.asarray(pred_boxes, np.float64)                   # [B,Q,4]
    tb = np.asarray(boxes_padded, np.float64)                 # [B,T,4]
    nb = np.asarray(num_boxes).astype(np.int64)               # [B]

    cx, cy, w, h = pb[..., 0], pb[..., 1], pb[..., 2], pb[..., 3]
    x0q, y0q = cx - 0.5 * w, cy - 0.5 * h
    x1q, y1q = cx + 0.5 * w, cy + 0.5 * h
    area1 = (x1q - x0q) * (y1q - y0q)

    p = 1.0 / (1.0 + np.exp(-pl))
    log_p = -np.log1p(np.exp(-pl))
    log_1mp = -np.log1p(np.exp(pl))
    cc = -0.25 * (1.0 - p) ** 2 * log_p + 0.75 * p ** 2 * log_1mp
    cc2 = 2.0 * cc + 2.0

    tcx, tcy, tw, th = tb[..., 0], tb[..., 1], tb[..., 2], tb[..., 3]
    tx0, ty0 = tcx - 0.5 * tw, tcy - 0.5 * th
    tx1, ty1 = tcx + 0.5 * tw, tcy + 0.5 * th
    area2 = (tx1 - tx0) * (ty1 - ty0)
    mk = (np.arange(T)[None, :] < nb[:, None]).astype(np.float64)   # [B,T]
    fill = np.full((B, T), INVALID, np.float64)

    trows = np.stack([tx0, ty0, tx1, ty1, tcx, tcy, tw, th, area2, mk, fill],
                     axis=1)                                   # [B,NREP,T]
    qcols = np.stack([x0q, y0q, x1q, y1q, cx, cy, w, h, area1, cc2],
                     axis=2)                                   # [B,Q,NSC]

    in_maps = []
    for c in range(N_CORES):
        bs = [int(slots[j][c]) for j in range(B_PER)]
        trep = np.broadcast_to(trows[bs][:, :, None, :], (B_PER, NREP, 128, T))
        trep = np.ascontiguousarray(trep, dtype=np.float32)
        # tile qt covers q rows [q0, q0+128), last tile overlapping
        qb = qcols[bs]
        tiles = [qb[:, min(qt * QT, Q - QT):min(qt * QT, Q - QT) + QT, :]
                 for qt in range(NQT)]
        qc = np.stack(tiles, axis=1)                  # [B_PER, NQT, QT, NSC]
        qc = qc.transpose(0, 2, 1, 3)
        qc = np.ascontiguousarray(qc.reshape(B_PER, QT, NQT * NSC),
                                  dtype=np.float32)
        in_maps.append({"trep": trep, "qsc": qc})
    return in_maps


def kernel(pred_logits, pred_boxes, boxes_padded, num_boxes):
    global LAST_RESULTS
    from concourse.bass_utils import run_bass_kernel_spmd

    slots, W = _plan(num_boxes)
    in_maps = _host_prep(pred_logits, pred_boxes, boxes_padded, num_boxes, slots)
    nc = _PROG_CACHE.get(W)
    if nc is None:
        nc = _build_program(W)
        _PROG_CACHE[W] = nc
    trace = bool(int(os.environ.get("BHM_TRACE", "0")))
    res = run_bass_kernel_spmd(nc, in_maps, list(range(N_CORES)), trace=trace)
    LAST_RESULTS = res
    out = np.empty((B, Q, T), np.float32)
    for c in range(N_CORES):
        slab = np.asarray(res.results[c]["C"]).reshape(B_PER, Q, T)
        for j in range(B_PER):
            out[int(slots[j][c])] = slab[j]
    return out
